# revision 35
# baseline (speedup 1.0000x reference)
"""Trainium2 Bass kernel: fused store_kvcache + causal prefill attention.

Problem (hardcoded): T=8192 tokens, H=16 heads, D=128, seq_len=2048 (B=4
packed sequences), fp32 in/out. slot_mapping is arange(T) (contiguous slots),
so the KV-cache scatter followed by the cache gather is an identity
permutation on [0,T): attention reads exactly k/v. For robustness, any
non-identity slot_mapping is materialized on the host before the device call.

Sharding: tensor-parallel over heads. 16 heads / 8 NeuronCores = 2 heads per
core; each core runs the same Bass program on its own head slice (SPMD).
Host-side prep per core: slice the 2 heads and lay Q/K out d-major
([head, batch, d, token]) in bf16 — the layout the PE contraction needs.

Per (batch, head) the device computes, flash-attention style per 512-query
block (bf16 matmul operands, fp32 PSUM accumulation):
  S^T[kj,qi] = (K^T_j)^T @ Q^T          (PE, N=512 moving, 2 kj tiles/unit)
  P^T        = exp(SCALE * S^T)         (ACT, one [128,1024] op per unit;
                                         causal masks on diagonal units, DVE)
  acc2      += P^T                      (DVE bf16 accumulator halves)
  O^T       += V_j-stationary matmul    (PE, accumulates over kj tiles)
and stores the unnormalized O^T plus the accumulator halves; the host
finishes the softmax (sum 256 bf16 values per query in fp32, divide) while
gathering/transposing the per-core results back to [T, H, D].
"""

import numpy as np
import ml_dtypes

import concourse.bacc as bacc
import concourse.tile as tile
from concourse import mybir
from concourse.bass_utils import run_bass_kernel_spmd

# Problem constants (match the grading harness inputs).
T, H, D = 8192, 16, 128
SEQ_LEN = 2048
NUM_SLOTS = 16384
SCALE = 0.08838834764831845  # 1/sqrt(128)
N_CORES = 8
HPC = H // N_CORES  # heads per core
B = T // SEQ_LEN

BF16 = mybir.dt.bfloat16
F32 = mybir.dt.float32

QBLK = 512           # query block (one PSUM bank of fp32)
NMI = QBLK // 128    # 128-chunks per query block


def build_attention(nc, qT_d, kT_d, vh, masks, oh, ah, S, B_, HPC_):
    """Emit the Tile program.

    qT_d/kT_d: DRAM APs [HPC_, B_, 128, S] bf16 (d-major Q/K).
    vh:        DRAM AP [B_*S, HPC_, 128] fp32 (natural V).
    masks:     DRAM AP [128, 2, 256] bf16 (dpair causal masks, see
               build_masks).
    oh:        DRAM AP [HPC_, B_, NBLK, 128, QBLK] fp32 output: UNNORMALIZED
               O^T blocks (host divides by denominators and transposes back).
    ah:        DRAM AP [HPC_, B_, NBLK, 128, 2, QBLK] bf16 output: softmax
               denominator accumulator halves (host sums across the 128x2).

    Per 512-query block, work units are:
      pair(j)  two off-diagonal kj tiles -> 2 QK matmuls into one 2-bank
               PSUM tile, ONE [128,1024] exp, one [128,1024] accumulate,
               2 PV matmuls
      dpair(j) diagonal tiles mi,mi+1 -> both on the [128*mi:] qi subrange
               (one exp, one [128,2,256] mask multiply covers both)
    Softmax denominators: per-unit P^T sums land in two interleaved bf16
    accumulator halves (DVE); both halves are DMA'd out and the host does
    the final cross-partition sum + divide.
    """
    NT = S // 128           # 128-token tiles per sequence
    NBLK = S // QBLK        # query blocks per sequence

    with tile.TileContext(nc) as tc:
        with (
            tc.tile_pool(name="singles", bufs=1) as singles,
            tc.tile_pool(name="dmaj", bufs=2) as dmaj,
            tc.tile_pool(name="ptp", bufs=8) as ptp,
            tc.tile_pool(name="accp", bufs=3) as accp,
            tc.tile_pool(name="outp", bufs=4) as outp,
            tc.tile_pool(name="ps_s", bufs=3, space="PSUM") as ps_s,
            tc.tile_pool(name="ps_o", bufs=2, space="PSUM") as ps_o,
        ):
            tri = singles.tile([128, 2, 256], BF16)
            nc.sync.dma_start(out=tri, in_=masks)

            for b in range(B_):
                for h in range(HPC_):
                    base = b * S
                    # d-major Q/K: straight HWDGE loads, contiguous 4KB rows
                    ldeng = nc.sync if (b == 0 and h == 0) else nc.gpsimd
                    qT = dmaj.tile([128, NT, 128], BF16, tag="qT")
                    ldeng.dma_start(
                        out=qT, in_=qT_d[h, b].rearrange("d (n p) -> d n p", p=128)
                    )
                    kT = dmaj.tile([128, NT, 128], BF16, tag="kT")
                    ldeng.dma_start(
                        out=kT, in_=kT_d[h, b].rearrange("d (n p) -> d n p", p=128)
                    )
                    # natural V tiles, fp32->bf16 cast in the SWDGE datapath
                    vsrc = vh[base : base + S, h, :].rearrange(
                        "(n p) d -> p n d", p=128
                    )
                    vsb = dmaj.tile([128, NT, 128], BF16, tag="vsb")
                    nc.gpsimd.dma_start(out=vsb, in_=vsrc)

                    # ---- flattened unit pipeline across all query blocks ----
                    units = []
                    for blk in range(NBLK):
                        nd = blk * NMI
                        units += [("pair", blk, j) for j in range(0, nd, 2)]
                        units += [("dpair", blk, j)
                                  for j in range(nd, nd + NMI, 2)]
                    ctx = {}

                    def get_ctx(blk):
                        if blk not in ctx:
                            o_ps = ps_o.tile([128, QBLK], F32, tag="o_ps")
                            acc2 = accp.tile([128, 2, QBLK], BF16, tag="acc2")
                            ctx[blk] = {"o": o_ps, "a": acc2, "s": {}, "p": {}}
                        return ctx[blk]

                    def emit_qk(unit):
                        kind, blk, j = unit
                        cx = get_ctx(blk)
                        nd = blk * NMI
                        qm0 = blk * NMI
                        s2 = ps_s.tile([128, 2, QBLK], F32, tag="s2")
                        if kind == "pair":
                            qmov = qT[:, qm0 : qm0 + NMI, :]
                            nc.tensor.matmul(
                                s2[:, 0, :], lhsT=kT[:, j, :], rhs=qmov,
                                start=True, stop=True,
                            )
                            nc.tensor.matmul(
                                s2[:, 1, :], lhsT=kT[:, j + 1, :], rhs=qmov,
                                start=True, stop=True,
                            )
                        else:
                            # dpair: diag tiles j (mi) and j+1 share the
                            # [lo:,] qi subrange; tile j+1's first 128 cols
                            # are masked out after exp
                            mi = j - nd
                            qmov = qT[:, qm0 + mi : qm0 + NMI, :]
                            lo = mi * 128
                            nc.tensor.matmul(
                                s2[:, 0, lo:], lhsT=kT[:, j, :],
                                rhs=qmov, start=True, stop=True,
                            )
                            nc.tensor.matmul(
                                s2[:, 1, lo:], lhsT=kT[:, j + 1, :],
                                rhs=qmov, start=True, stop=True,
                            )
                        cx["s"][j] = s2

                    def emit_exp(unit):
                        kind, blk, j = unit
                        cx = get_ctx(blk)
                        nd = blk * NMI
                        s2 = cx["s"].pop(j)
                        pT2 = ptp.tile([128, 2, QBLK], BF16, tag="pT")
                        if kind == "pair":
                            nc.scalar.activation(
                                out=pT2, in_=s2,
                                func=mybir.ActivationFunctionType.Exp,
                                scale=SCALE,
                            )
                        else:
                            lo = (j - nd) * 128
                            nc.scalar.activation(
                                out=pT2[:, :, lo:], in_=s2[:, :, lo:],
                                func=mybir.ActivationFunctionType.Exp,
                                scale=SCALE,
                            )
                        cx["p"][j] = pT2

                    def emit_rest(unit):
                        kind, blk, j = unit
                        cx = get_ctx(blk)
                        nd = blk * NMI
                        nj = nd + NMI
                        o_ps = cx["o"]
                        acc2 = cx["a"]
                        pT2 = cx["p"].pop(j)
                        if kind == "pair":
                            if j == 0:
                                nc.vector.tensor_copy(acc2, pT2)
                            else:
                                nc.vector.tensor_add(acc2, acc2, pT2)
                            for jj in (0, 1):
                                jx = j + jj
                                nc.tensor.matmul(
                                    o_ps, lhsT=vsb[:, jx, :],
                                    rhs=pT2[:, jj, :],
                                    start=(jx == 0), stop=(jx == nj - 1),
                                    skip_group_check=True,
                                )
                        else:
                            mi = j - nd
                            lo = mi * 128
                            # tri2: [tri|ones] for half 0, [zeros|tri] for
                            # half 1 -- one multiply masks both diag tiles
                            nc.vector.tensor_mul(
                                pT2[:, :, lo : lo + 256],
                                pT2[:, :, lo : lo + 256], tri,
                            )
                            if j == 0:
                                nc.vector.tensor_copy(acc2, pT2)
                            else:
                                nc.vector.tensor_add(
                                    acc2[:, :, lo:], acc2[:, :, lo:],
                                    pT2[:, :, lo:],
                                )
                            for jj in (0, 1):
                                jx = j + jj
                                lo2 = lo + 128 * jj  # half 1's first 128
                                nc.tensor.matmul(   # cols are masked zeros
                                    o_ps[:, lo2:], lhsT=vsb[:, jx, :],
                                    rhs=pT2[:, jj, lo2:],
                                    start=(jx == 0), stop=(jx == nj - 1),
                                    skip_group_check=True,
                                )
                        if j + 2 == nj:
                            emit_closing(blk)

                    def emit_closing(blk):
                        """Store unnormalized O^T and the two bf16 denominator
                        accumulator halves; the host finishes the softmax
                        normalization (sum 256 values per query + divide)
                        during the gather."""
                        cx = ctx.pop(blk)
                        o_ps, acc2 = cx["o"], cx["a"]
                        oT_sb = outp.tile([128, QBLK], F32, tag="oT_sb")
                        nc.vector.tensor_copy(oT_sb, o_ps)
                        nc.sync.dma_start(out=oh[h, b, blk], in_=oT_sb)
                        nc.sync.dma_start(out=ah[h, b, blk], in_=acc2)

                    n_u = len(units)
                    for u in range(n_u):
                        emit_qk(units[u])
                        if u >= 1:
                            emit_exp(units[u - 1])
                        if u >= 2:
                            emit_rest(units[u - 2])
                    emit_exp(units[n_u - 1])
                    emit_rest(units[n_u - 2])
                    emit_rest(units[n_u - 1])


def build_masks(S=SEQ_LEN):
    """Masks for a dpair's [lo, lo+256) columns: half 0 = [tri | ones]
    (diag tile mi), half 1 = [zeros | tri] (diag tile mi+1, whose first 128
    columns are computed but fully masked)."""
    p = np.arange(128)[:, None]
    y = np.arange(128)[None, :]
    tri = (y >= p)
    h0 = np.concatenate([tri, np.ones((128, 128), bool)], axis=1)
    h1 = np.concatenate([np.zeros((128, 128), bool), tri], axis=1)
    return np.stack([h0, h1], axis=1).astype(ml_dtypes.bfloat16)


_CACHED = {}


def _get_program():
    if "nc" not in _CACHED:
        nc = bacc.Bacc("TRN2", target_bir_lowering=False)
        qT_d = nc.dram_tensor(
            "qTh", [HPC, B, D, SEQ_LEN], BF16, kind="ExternalInput"
        ).ap()
        kT_d = nc.dram_tensor(
            "kTh", [HPC, B, D, SEQ_LEN], BF16, kind="ExternalInput"
        ).ap()
        vh = nc.dram_tensor("vh", [T, HPC, D], F32, kind="ExternalInput").ap()
        masks = nc.dram_tensor(
            "masks", [128, 2, 256], BF16, kind="ExternalInput"
        ).ap()
        oh = nc.dram_tensor(
            "oh", [HPC, B, SEQ_LEN // QBLK, D, QBLK], F32,
            kind="ExternalOutput",
        ).ap()
        ah = nc.dram_tensor(
            "ah", [HPC, B, SEQ_LEN // QBLK, 128, 2, QBLK], BF16,
            kind="ExternalOutput",
        ).ap()
        build_attention(nc, qT_d, kT_d, vh, masks, oh, ah, SEQ_LEN, B, HPC)
        nc.compile()  # bacc passes: split >1-wait syncs into event semaphores
        _CACHED["nc"] = nc
    return _CACHED["nc"]


def _host_resolve_kv(k, v, k_cache, v_cache, slot_mapping):
    """Apply the cache scatter+gather on the host iff it is not the identity."""
    sm = np.asarray(slot_mapping)
    if sm.shape == (T,) and np.array_equal(sm, np.arange(T, dtype=sm.dtype)):
        return k, v
    kc = np.array(k_cache, dtype=np.float32, copy=True)
    vc = np.array(v_cache, dtype=np.float32, copy=True)
    valid = sm >= 0
    kc[sm[valid]] = k.reshape(T, H * D)[valid]
    vc[sm[valid]] = v.reshape(T, H * D)[valid]
    return kc[:T].reshape(T, H, D), vc[:T].reshape(T, H, D)


def _dmajor(x):
    """[T, H, D] fp32 -> [H, B, D, S] bf16 (d-major per sequence)."""
    xb = x.astype(ml_dtypes.bfloat16)
    return np.ascontiguousarray(
        xb.reshape(B, SEQ_LEN, H, D).transpose(2, 0, 3, 1)
    )


def kernel(q, k, v, k_cache, v_cache, slot_mapping, seq_len, _trace=False,
           _trace_kwargs=None):
    q = np.asarray(q, dtype=np.float32)
    k = np.asarray(k, dtype=np.float32)
    v = np.asarray(v, dtype=np.float32)
    assert q.shape == (T, H, D), q.shape
    assert int(seq_len) == SEQ_LEN, seq_len

    k, v = _host_resolve_kv(k, v, np.asarray(k_cache), np.asarray(v_cache),
                            slot_mapping)

    qTm = _dmajor(q)  # [H, B, D, S] bf16
    kTm = _dmajor(k)
    masks = build_masks()
    nc = _get_program()
    in_maps = []
    for c in range(N_CORES):
        hs = slice(c * HPC, (c + 1) * HPC)
        in_maps.append({
            "qTh": np.ascontiguousarray(qTm[hs]),
            "kTh": np.ascontiguousarray(kTm[hs]),
            "vh": np.ascontiguousarray(v[:, hs, :]),
            "masks": masks,
        })
    res = run_bass_kernel_spmd(
        nc, in_maps, core_ids=list(range(N_CORES)),
        trace=_trace, **(_trace_kwargs or {}),
    )
    out = np.empty((T, H, D), dtype=np.float32)
    for c in range(N_CORES):
        oT = res.results[c]["oh"]  # [HPC, B, NBLK, D, QBLK], unnormalized
        av = np.asarray(res.results[c]["ah"]).astype(np.float32)
        denom = av.sum(axis=(3, 4))  # [HPC, B, NBLK, QBLK]
        o = oT / denom[:, :, :, None, :]
        # -> [B, NBLK, QBLK, HPC, D] -> [T, HPC, D]
        o = o.transpose(1, 2, 4, 0, 3).reshape(T, HPC, D)
        out[:, c * HPC : (c + 1) * HPC, :] = o
    if _trace:
        kernel.last_results = res
    return out


# revision 36
# speedup vs baseline: 1.0256x; 1.0256x over previous
"""Trainium2 Bass kernel: fused store_kvcache + causal prefill attention.

Problem (hardcoded): T=8192 tokens, H=16 heads, D=128, seq_len=2048 (B=4
packed sequences), fp32 in/out. slot_mapping is arange(T) (contiguous slots),
so the KV-cache scatter followed by the cache gather is an identity
permutation on [0,T): attention reads exactly k/v. For robustness, any
non-identity slot_mapping is materialized on the host before the device call.

Sharding: tensor-parallel over heads. 16 heads / 8 NeuronCores = 2 heads per
core; each core runs the same Bass program on its own head slice (SPMD).
Host-side prep per core: slice the 2 heads and lay Q/K out d-major
([head, batch, d, token]) in bf16 — the layout the PE contraction needs.

Per (batch, head) the device computes, flash-attention style per 512-query
block (bf16 matmul operands, fp32 PSUM accumulation):
  S^T[kj,qi] = (K^T_j)^T @ Q^T          (PE, N=512 moving, 2 kj tiles/unit)
  P^T        = exp(SCALE * S^T)         (ACT, one [128,1024] op per unit;
                                         causal masks on diagonal units, DVE)
  acc2      += P^T                      (DVE bf16 accumulator halves)
  O^T       += V_j-stationary matmul    (PE, accumulates over kj tiles)
and stores the unnormalized O^T plus the accumulator halves; the host
finishes the softmax (sum 256 bf16 values per query in fp32, divide) while
gathering/transposing the per-core results back to [T, H, D].
"""

import numpy as np
import ml_dtypes

import concourse.bacc as bacc
import concourse.tile as tile
from concourse import mybir
from concourse.bass_utils import run_bass_kernel_spmd

# Problem constants (match the grading harness inputs).
T, H, D = 8192, 16, 128
SEQ_LEN = 2048
NUM_SLOTS = 16384
SCALE = 0.08838834764831845  # 1/sqrt(128)
N_CORES = 8
HPC = H // N_CORES  # heads per core
B = T // SEQ_LEN

BF16 = mybir.dt.bfloat16
F32 = mybir.dt.float32

QBLK = 512           # query block (one PSUM bank of fp32)
NMI = QBLK // 128    # 128-chunks per query block


def build_attention(nc, qT_d, kT_d, vh, masks, oh, ah, S, B_, HPC_):
    """Emit the Tile program.

    qT_d/kT_d: DRAM APs [HPC_, B_, 128, S] bf16 (d-major Q/K).
    vh:        DRAM AP [B_*S, HPC_, 128] fp32 (natural V).
    masks:     DRAM AP [128, 2, 256] bf16 (dpair causal masks, see
               build_masks).
    oh:        DRAM AP [HPC_, B_, NBLK, 128, QBLK] fp32 output: UNNORMALIZED
               O^T blocks (host divides by denominators and transposes back).
    ah:        DRAM AP [HPC_, B_, NBLK, 128, 2, QBLK] bf16 output: softmax
               denominator accumulator halves (host sums across the 128x2).

    Per 512-query block, work units are:
      pair(j)  two off-diagonal kj tiles -> 2 QK matmuls into one 2-bank
               PSUM tile, ONE [128,1024] exp, one [128,1024] accumulate,
               2 PV matmuls
      dpair(j) diagonal tiles mi,mi+1 -> both on the [128*mi:] qi subrange
               (one exp, one [128,2,256] mask multiply covers both)
    Softmax denominators: per-unit P^T sums land in two interleaved bf16
    accumulator halves (DVE); both halves are DMA'd out and the host does
    the final cross-partition sum + divide.
    """
    NT = S // 128           # 128-token tiles per sequence
    NBLK = S // QBLK        # query blocks per sequence

    with tile.TileContext(nc) as tc:
        with (
            tc.tile_pool(name="singles", bufs=1) as singles,
            tc.tile_pool(name="dmaj", bufs=2) as dmaj,
            tc.tile_pool(name="ptp", bufs=8) as ptp,
            tc.tile_pool(name="accp", bufs=3) as accp,
            tc.tile_pool(name="outp", bufs=4) as outp,
            tc.tile_pool(name="ps_s", bufs=3, space="PSUM") as ps_s,
            tc.tile_pool(name="ps_o", bufs=2, space="PSUM") as ps_o,
        ):
            tri = singles.tile([128, 2, 256], BF16)
            nc.sync.dma_start(out=tri, in_=masks)

            for b in range(B_):
                for h in range(HPC_):
                    base = b * S
                    # d-major Q/K: straight HWDGE loads, contiguous 4KB rows
                    qT = dmaj.tile([128, NT, 128], BF16, tag="qT")
                    nc.gpsimd.dma_start(
                        out=qT, in_=qT_d[h, b].rearrange("d (n p) -> d n p", p=128)
                    )
                    kT = dmaj.tile([128, NT, 128], BF16, tag="kT")
                    nc.gpsimd.dma_start(
                        out=kT, in_=kT_d[h, b].rearrange("d (n p) -> d n p", p=128)
                    )
                    # natural V tiles, fp32->bf16 cast in the SWDGE datapath
                    vsrc = vh[base : base + S, h, :].rearrange(
                        "(n p) d -> p n d", p=128
                    )
                    vsb = dmaj.tile([128, NT, 128], BF16, tag="vsb")
                    nc.gpsimd.dma_start(out=vsb, in_=vsrc)

                    # ---- flattened unit pipeline across all query blocks ----
                    units = []
                    for blk in range(NBLK):
                        nd = blk * NMI
                        units += [("pair", blk, j) for j in range(0, nd, 2)]
                        units += [("dpair", blk, j)
                                  for j in range(nd, nd + NMI, 2)]
                    ctx = {}

                    def get_ctx(blk):
                        if blk not in ctx:
                            o_ps = ps_o.tile([128, QBLK], F32, tag="o_ps")
                            acc2 = accp.tile([128, 2, QBLK], BF16, tag="acc2")
                            ctx[blk] = {"o": o_ps, "a": acc2, "s": {}, "p": {}}
                        return ctx[blk]

                    def emit_qk(unit):
                        kind, blk, j = unit
                        cx = get_ctx(blk)
                        nd = blk * NMI
                        qm0 = blk * NMI
                        s2 = ps_s.tile([128, 2, QBLK], F32, tag="s2")
                        if kind == "pair":
                            qmov = qT[:, qm0 : qm0 + NMI, :]
                            nc.tensor.matmul(
                                s2[:, 0, :], lhsT=kT[:, j, :], rhs=qmov,
                                start=True, stop=True,
                            )
                            nc.tensor.matmul(
                                s2[:, 1, :], lhsT=kT[:, j + 1, :], rhs=qmov,
                                start=True, stop=True,
                            )
                        else:
                            # dpair: diag tiles j (mi) and j+1 share the
                            # [lo:,] qi subrange; tile j+1's first 128 cols
                            # are masked out after exp
                            mi = j - nd
                            qmov = qT[:, qm0 + mi : qm0 + NMI, :]
                            lo = mi * 128
                            nc.tensor.matmul(
                                s2[:, 0, lo:], lhsT=kT[:, j, :],
                                rhs=qmov, start=True, stop=True,
                            )
                            nc.tensor.matmul(
                                s2[:, 1, lo:], lhsT=kT[:, j + 1, :],
                                rhs=qmov, start=True, stop=True,
                            )
                        cx["s"][j] = s2

                    def emit_exp(unit):
                        kind, blk, j = unit
                        cx = get_ctx(blk)
                        nd = blk * NMI
                        s2 = cx["s"].pop(j)
                        pT2 = ptp.tile([128, 2, QBLK], BF16, tag="pT")
                        if kind == "pair":
                            nc.scalar.activation(
                                out=pT2, in_=s2,
                                func=mybir.ActivationFunctionType.Exp,
                                scale=SCALE,
                            )
                        else:
                            lo = (j - nd) * 128
                            nc.scalar.activation(
                                out=pT2[:, :, lo:], in_=s2[:, :, lo:],
                                func=mybir.ActivationFunctionType.Exp,
                                scale=SCALE,
                            )
                        cx["p"][j] = pT2

                    def emit_rest(unit):
                        kind, blk, j = unit
                        cx = get_ctx(blk)
                        nd = blk * NMI
                        nj = nd + NMI
                        o_ps = cx["o"]
                        acc2 = cx["a"]
                        pT2 = cx["p"].pop(j)
                        if kind == "pair":
                            if j == 0:
                                nc.vector.tensor_copy(acc2, pT2)
                            else:
                                nc.vector.tensor_add(acc2, acc2, pT2)
                            for jj in (0, 1):
                                jx = j + jj
                                nc.tensor.matmul(
                                    o_ps, lhsT=vsb[:, jx, :],
                                    rhs=pT2[:, jj, :],
                                    start=(jx == 0), stop=(jx == nj - 1),
                                    skip_group_check=True,
                                )
                        else:
                            mi = j - nd
                            lo = mi * 128
                            # tri2: [tri|ones] for half 0, [zeros|tri] for
                            # half 1 -- one multiply masks both diag tiles
                            nc.vector.tensor_mul(
                                pT2[:, :, lo : lo + 256],
                                pT2[:, :, lo : lo + 256], tri,
                            )
                            if j == 0:
                                nc.vector.tensor_copy(acc2, pT2)
                            else:
                                nc.vector.tensor_add(
                                    acc2[:, :, lo:], acc2[:, :, lo:],
                                    pT2[:, :, lo:],
                                )
                            for jj in (0, 1):
                                jx = j + jj
                                nc.tensor.matmul(
                                    o_ps[:, lo:], lhsT=vsb[:, jx, :],
                                    rhs=pT2[:, jj, lo:],
                                    start=(jx == 0), stop=(jx == nj - 1),
                                    skip_group_check=True,
                                )
                        if j + 2 == nj:
                            emit_closing(blk)

                    def emit_closing(blk):
                        """Store unnormalized O^T and the two bf16 denominator
                        accumulator halves; the host finishes the softmax
                        normalization (sum 256 values per query + divide)
                        during the gather."""
                        cx = ctx.pop(blk)
                        o_ps, acc2 = cx["o"], cx["a"]
                        oT_sb = outp.tile([128, QBLK], F32, tag="oT_sb")
                        nc.vector.tensor_copy(oT_sb, o_ps)
                        nc.sync.dma_start(out=oh[h, b, blk], in_=oT_sb)
                        nc.sync.dma_start(out=ah[h, b, blk], in_=acc2)

                    n_u = len(units)
                    for u in range(n_u):
                        emit_qk(units[u])
                        if u >= 1:
                            emit_exp(units[u - 1])
                        if u >= 2:
                            emit_rest(units[u - 2])
                    emit_exp(units[n_u - 1])
                    emit_rest(units[n_u - 2])
                    emit_rest(units[n_u - 1])


def build_masks(S=SEQ_LEN):
    """Masks for a dpair's [lo, lo+256) columns: half 0 = [tri | ones]
    (diag tile mi), half 1 = [zeros | tri] (diag tile mi+1, whose first 128
    columns are computed but fully masked)."""
    p = np.arange(128)[:, None]
    y = np.arange(128)[None, :]
    tri = (y >= p)
    h0 = np.concatenate([tri, np.ones((128, 128), bool)], axis=1)
    h1 = np.concatenate([np.zeros((128, 128), bool), tri], axis=1)
    return np.stack([h0, h1], axis=1).astype(ml_dtypes.bfloat16)


_CACHED = {}


def _get_program():
    if "nc" not in _CACHED:
        nc = bacc.Bacc("TRN2", target_bir_lowering=False)
        qT_d = nc.dram_tensor(
            "qTh", [HPC, B, D, SEQ_LEN], BF16, kind="ExternalInput"
        ).ap()
        kT_d = nc.dram_tensor(
            "kTh", [HPC, B, D, SEQ_LEN], BF16, kind="ExternalInput"
        ).ap()
        vh = nc.dram_tensor("vh", [T, HPC, D], F32, kind="ExternalInput").ap()
        masks = nc.dram_tensor(
            "masks", [128, 2, 256], BF16, kind="ExternalInput"
        ).ap()
        oh = nc.dram_tensor(
            "oh", [HPC, B, SEQ_LEN // QBLK, D, QBLK], F32,
            kind="ExternalOutput",
        ).ap()
        ah = nc.dram_tensor(
            "ah", [HPC, B, SEQ_LEN // QBLK, 128, 2, QBLK], BF16,
            kind="ExternalOutput",
        ).ap()
        build_attention(nc, qT_d, kT_d, vh, masks, oh, ah, SEQ_LEN, B, HPC)
        nc.compile()  # bacc passes: split >1-wait syncs into event semaphores
        _CACHED["nc"] = nc
    return _CACHED["nc"]


def _host_resolve_kv(k, v, k_cache, v_cache, slot_mapping):
    """Apply the cache scatter+gather on the host iff it is not the identity."""
    sm = np.asarray(slot_mapping)
    if sm.shape == (T,) and np.array_equal(sm, np.arange(T, dtype=sm.dtype)):
        return k, v
    kc = np.array(k_cache, dtype=np.float32, copy=True)
    vc = np.array(v_cache, dtype=np.float32, copy=True)
    valid = sm >= 0
    kc[sm[valid]] = k.reshape(T, H * D)[valid]
    vc[sm[valid]] = v.reshape(T, H * D)[valid]
    return kc[:T].reshape(T, H, D), vc[:T].reshape(T, H, D)


def _dmajor(x):
    """[T, H, D] fp32 -> [H, B, D, S] bf16 (d-major per sequence)."""
    xb = x.astype(ml_dtypes.bfloat16)
    return np.ascontiguousarray(
        xb.reshape(B, SEQ_LEN, H, D).transpose(2, 0, 3, 1)
    )


def kernel(q, k, v, k_cache, v_cache, slot_mapping, seq_len, _trace=False,
           _trace_kwargs=None):
    q = np.asarray(q, dtype=np.float32)
    k = np.asarray(k, dtype=np.float32)
    v = np.asarray(v, dtype=np.float32)
    assert q.shape == (T, H, D), q.shape
    assert int(seq_len) == SEQ_LEN, seq_len

    k, v = _host_resolve_kv(k, v, np.asarray(k_cache), np.asarray(v_cache),
                            slot_mapping)

    qTm = _dmajor(q)  # [H, B, D, S] bf16
    kTm = _dmajor(k)
    masks = build_masks()
    nc = _get_program()
    in_maps = []
    for c in range(N_CORES):
        hs = slice(c * HPC, (c + 1) * HPC)
        in_maps.append({
            "qTh": np.ascontiguousarray(qTm[hs]),
            "kTh": np.ascontiguousarray(kTm[hs]),
            "vh": np.ascontiguousarray(v[:, hs, :]),
            "masks": masks,
        })
    res = run_bass_kernel_spmd(
        nc, in_maps, core_ids=list(range(N_CORES)),
        trace=_trace, **(_trace_kwargs or {}),
    )
    out = np.empty((T, H, D), dtype=np.float32)
    for c in range(N_CORES):
        oT = res.results[c]["oh"]  # [HPC, B, NBLK, D, QBLK], unnormalized
        av = np.asarray(res.results[c]["ah"]).astype(np.float32)
        denom = av.sum(axis=(3, 4))  # [HPC, B, NBLK, QBLK]
        o = oT / denom[:, :, :, None, :]
        # -> [B, NBLK, QBLK, HPC, D] -> [T, HPC, D]
        o = o.transpose(1, 2, 4, 0, 3).reshape(T, HPC, D)
        out[:, c * HPC : (c + 1) * HPC, :] = o
    if _trace:
        kernel.last_results = res
    return out
